# revision 35
# baseline (speedup 1.0000x reference)
"""Multi-head self-attention Bass/Tile kernel for Trainium2 (8 NeuronCores).

Problem: B=16, N=1024, input/embed dim 128, 8 heads x head_dim 16.
  Q = q @ Wq_h; K = q @ Wk_h; V = q @ Wv_h   (per head)
  S = norm * Q K^T, masked softmax over keys, out = sum_h (A_h V_h) @ Wo_h

Sharding: data-parallel over batch, 2 batches per core, no collectives.

Per-core design (transposed "S^T" layout: softmaxed weights live as
[key m on partitions, query n on free] so A@V needs no transposes):

  * host sends qT [d, n] fp16 and maskT [m, n] bf16; per-head weights
    packed into 32-row-strip group layouts (3/3/2 heads per group).
  * projections: QT_g/KT_g [128, n] fp16 with head j of a group at
    partition strip 32j; V in natural [m, (h,v)] bf16 with an appended
    ones column per head (computes the softmax denominator during A@V).
  * steady loop per (chunk c, group g): for each key m-tile:
    scores via row-tiled fp16 matmuls (K=16, concurrent in 32-row PE
    strips) into one multi-bank PSUM tile [128, 512*nh]; one ScalarE
    exp (PSUM->SBUF bf16); one VectorE mask multiply; A@V via col-tiled
    fp16 matmuls (M=17) accumulating into a single PSUM bank using
    start=True has_written semantics (no per-block memset).
  * normalization: VectorE stream_shuffle broadcasts each strip's
    denominator row (local row 16) to all 32 rows of its quadrant in
    one op, reciprocal, then one multiply into headsT (fp16).
  * output projection transposed: outT[e, n] = sum_g Wo_g^T @ headsT_g
    (3 accumulating matmuls per 512-chunk); host transposes back.
  * software pipelining: the last A@V + normalization + out-projection
    of a block are emitted after the next block's first scores/exp so
    the ScalarE exp stream (the bottleneck) never stalls; batch 1's
    projections are interleaved into batch 0's blocks.
"""

import os
import sys

sys.path.insert(0, "/opt/trn_rl_repo")

import numpy as np

B, N, D, H, HD = 16, 1024, 128, 8, 16
NCORES = 8
BPC = B // NCORES  # batches per core
NORM = 1.0 / np.sqrt(HD)
GROUPS = [(0, 1, 2), (3, 4, 5), (6, 7)]
NCHUNK = 512  # query free-dim chunk (one PSUM bank)
MT = N // 128  # key m-tiles per batch
VSTRIDE = HD + 1  # V columns per head incl. ones column

SIM_MODE = False  # kept for harness compatibility; unused

_built = {}


def _build_nc():
    import concourse.mybir as mybir
    from concourse import bacc
    from concourse.tile import TileContext

    f32 = mybir.dt.float32
    f16 = mybir.dt.float16
    bf16 = mybir.dt.bfloat16
    AF = mybir.ActivationFunctionType

    nc = bacc.Bacc()

    qt_d = nc.dram_tensor("qt", [BPC, D, N], f16, kind="ExternalInput")
    mk_d = nc.dram_tensor("maskt", [BPC, N, N], bf16, kind="ExternalInput")
    wq_d = nc.dram_tensor("wq", [3, D, 128], f16, kind="ExternalInput")
    wk_d = nc.dram_tensor("wk", [3, D, 128], f16, kind="ExternalInput")
    wv_d = nc.dram_tensor("wv", [D, H * VSTRIDE], f16, kind="ExternalInput")
    wo_d = nc.dram_tensor("wo", [3, 128, D], f16, kind="ExternalInput")
    out_d = nc.dram_tensor("outT", [BPC, D, N], f32, kind="ExternalOutput")

    with TileContext(nc) as tc:
        with (
            tc.sbuf_pool(name="consts", bufs=1) as consts,
            tc.sbuf_pool(name="perbatch", bufs=2) as pb,
            tc.sbuf_pool(name="epool", bufs=6) as ep,
            tc.sbuf_pool(name="dpool", bufs=2) as dp,
            tc.sbuf_pool(name="spool", bufs=2) as sp,
            tc.psum_pool(name="pscore", bufs=2) as pscore,
            tc.psum_pool(name="pwork", bufs=2) as pwork,
        ):
            # exp table preload: make the first ACTIVATE (and its ~2.7us
            # table load) happen during the initial DMAs, off the
            # critical path.
            warm = consts.tile([1, 16], f32, name="warm")
            nc.vector.memset(warm, 0.0)
            nc.scalar.activation(warm, warm, AF.Exp)

            # zero row for the K=1 bank-zeroing matmuls
            zc = consts.tile([1, NCHUNK], bf16, name="zc")
            nc.vector.memset(zc, 0.0)

            # ---------- per-batch state ----------
            qt_sb = {}
            mask_sb = {}
            qtg = {}
            ktg = {}
            vaug = {}
            headsT = {}

            def emit_qt(b):
                qt_sb[b] = pb.tile([D, N], f16, name="qt_sb")
                for h in range(2):
                    hs = slice(h * (N // 2), (h + 1) * (N // 2))
                    nc.sync.dma_start(qt_sb[b][:, hs], qt_d[b][:, hs])

            def emit_masks(b):
                mask_sb[b] = pb.tile([128, MT * N], bf16, name="mask_sb")
                for mt in range(MT):
                    nc.sync.dma_start(
                        mask_sb[b][:, mt * N : (mt + 1) * N],
                        mk_d[b, mt * 128 : (mt + 1) * 128, :],
                    )

            def emit_dmas(b):
                emit_qt(b)
                emit_masks(b)

            # --- weights: everything on the critical path to the first
            # exp (g0 weights, qt, wv) clears the sync queue before the
            # bulky mask transfers; remaining weights follow.
            wq_sb, wk_sb, wo_sb = [None] * 3, [None] * 3, [None] * 3

            def load_w(lst, idx, shape, name, src):
                t = consts.tile(shape, f16, name=name)
                nc.sync.dma_start(t, src)
                lst[idx] = t

            load_w(wq_sb, 0, [D, 128], "wq_sb0", wq_d[0])
            load_w(wk_sb, 0, [D, 128], "wk_sb0", wk_d[0])
            emit_qt(0)
            wv_sb = consts.tile([D, H * VSTRIDE], f16, name="wv_sb")
            nc.sync.dma_start(wv_sb, wv_d[:, :])
            emit_masks(0)
            for g in (1, 2):
                load_w(wq_sb, g, [D, 128], f"wq_sb{g}", wq_d[g])
                load_w(wk_sb, g, [D, 128], f"wk_sb{g}", wk_d[g])
            for g in range(3):
                load_w(wo_sb, g, [128, D], f"wo_sb{g}", wo_d[g])

            # PE warm-up during the input DMA wait: a few long matmuls
            # flip the HAM clock gate to 8/8 so the projection matmuls
            # and first blocks run at 2.4 GHz.
            wup = pwork.tile([128, NCHUNK], f32, name="wup", tag="w")
            for _ in range(6):
                nc.tensor.matmul(
                    wup, lhsT=zc[:, :128], rhs=zc[:, :], start=True, stop=True
                )

            def projqk_pieces(b, g, use_scalar):
                # Q/K projections for group g as granular (matmul, cast)
                # pieces so the work queue can spread them one per
                # m-tile iteration. use_scalar routes the Q casts to
                # ScalarE when it would otherwise idle (batch 0 head).
                def alloc():
                    if g == 0:
                        qtg[b] = {}
                        ktg[b] = {}
                    qtg[b][g] = pb.tile([128, N], f16, name=f"qtg{g}")
                    ktg[b][g] = pb.tile([128, N], f16, name=f"ktg{g}")

                def one(c, which):
                    cs = slice(c * NCHUNK, (c + 1) * NCHUNK)
                    if which == "q":
                        ps = pscore.tile([128, NCHUNK], f32, name="ps_q", tag="sc")
                        nc.tensor.matmul(
                            ps, lhsT=wq_sb[g], rhs=qt_sb[b][:, cs],
                            start=True, stop=True,
                        )
                        if use_scalar:
                            nc.scalar.copy(qtg[b][g][:, cs], ps)
                        else:
                            nc.vector.tensor_copy(qtg[b][g][:, cs], ps)
                    else:
                        ps2 = pscore.tile([128, NCHUNK], f32, name="ps_k", tag="sc")
                        nc.tensor.matmul(
                            ps2, lhsT=wk_sb[g], rhs=qt_sb[b][:, cs],
                            start=True, stop=True,
                        )
                        nc.vector.tensor_copy(ktg[b][g][:, cs], ps2)

                pieces = [alloc]
                for c in range(N // NCHUNK):
                    pieces.append(lambda c=c: one(c, "q"))
                    pieces.append(lambda c=c: one(c, "k"))
                return pieces

            def projv_pieces(b):
                def alloc():
                    vaug[b] = pb.tile([128, MT * H * VSTRIDE], bf16, name="vaug")
                    vview = vaug[b].rearrange(
                        "p (mt h s) -> p mt h s", mt=MT, h=H, s=VSTRIDE
                    )
                    # ones columns first; the copies below skip them, so
                    # an A@V for m-tile mt depends only on copy(mt).
                    nc.vector.memset(vview[:, :, :, HD : HD + 1], 1.0)
                    headsT[b] = []
                    for g in range(3):
                        headsT[b].append(pb.tile([128, N], f16, name=f"headsT{g}"))

                def one(mt):
                    vview = vaug[b].rearrange(
                        "p (mt h s) -> p mt h s", mt=MT, h=H, s=VSTRIDE
                    )
                    ps = pscore.tile([128, NCHUNK], f32, name="ps_v", tag="sc")
                    nc.tensor.matmul(
                        ps[:, : H * VSTRIDE],
                        lhsT=qt_sb[b][:, mt * 128 : (mt + 1) * 128],
                        rhs=wv_sb,
                        start=True,
                        stop=True,
                    )
                    psv = ps[:, : H * VSTRIDE].rearrange(
                        "p (h s) -> p h s", h=H, s=VSTRIDE
                    )
                    nc.vector.tensor_copy(vview[:, mt, :, :HD], psv[:, :, :HD])

                return [alloc] + [lambda mt=mt: one(mt) for mt in range(MT)]

            # deferred work (last AV + normalization of the previous
            # block, out-projection of the previous chunk), emitted
            # Global lag-2 A@V pipeline: the A@V emitted in an iteration
            # is the one from TWO iterations back, so the mask multiply
            # it depends on is two exp-periods old and never stalls the
            # in-order PE queue (which would head-of-line-block the next
            # scores and starve the exp stream). avq carries the
            # not-yet-emitted A@V closures across block boundaries.
            avq = []

            def pump_avq():
                if len(avq) > 2:
                    avq.pop(0)()

            def emit_block(b, c, g):
                heads = GROUPS[g]
                nh = len(heads)
                cs = slice(c * NCHUNK, (c + 1) * NCHUNK)
                av = pwork.tile([128, NCHUNK], f32, name="av_ps", tag="w")
                # K=1 zeroing matmul: opens the bank's accumulation group
                # and initializes every element, so the col-tiled AV
                # matmuls below can accumulate (identical semantics on HW
                # and in CoreSim's pending-zero model).
                nc.tensor.matmul(
                    av, lhsT=zc[:, :128], rhs=zc[:, :], start=True, stop=False,
                    skip_group_check=True,
                )
                e_live = {}

                def make_av(mt):
                    def emit_av():
                        ep_ = e_live.pop(mt)
                        for j in range(nh):
                            h = heads[j]
                            nc.tensor.matmul(
                                av[32 * j : 32 * j + VSTRIDE, :],
                                lhsT=vaug[b][
                                    :,
                                    mt * H * VSTRIDE
                                    + h * VSTRIDE : mt * H * VSTRIDE
                                    + (h + 1) * VSTRIDE,
                                ],
                                rhs=ep_[:, j * NCHUNK : (j + 1) * NCHUNK],
                                start=False,
                                stop=(mt == MT - 1 and j == nh - 1),
                                skip_group_check=True,
                            )

                    return emit_av

                def emit_norm():
                    # evacuate av to SBUF (releases the PSUM slot), then
                    # broadcast each strip's denominator row (local row
                    # 16) across its 32-row quadrant via stream_shuffle,
                    # reciprocal, and scale into headsT.
                    ds = dp.tile([96, NCHUNK], f32, name="dsrc")
                    nc.vector.tensor_copy(ds, av[0:96, :])
                    dn = dp.tile([96, NCHUNK], f32, name="dnorm")
                    nc.vector.stream_shuffle(dn, ds, mask=[16] * 32)
                    nc.vector.reciprocal_approx_fast(dn, dn)
                    # all-SBUF multiply: run it on the otherwise-idle
                    # GpSimd to keep VectorE under the exp-period budget
                    nc.gpsimd.tensor_mul(headsT[b][g][0:96, cs], ds, dn)

                for mt in range(MT):
                    sc = pscore.tile([128, NCHUNK * nh], f32, name="sc", tag="sc")
                    for j in range(nh):
                        nc.tensor.matmul(
                            sc[:, j * NCHUNK : (j + 1) * NCHUNK],
                            lhsT=ktg[b][g][
                                32 * j : 32 * j + HD,
                                mt * 128 : (mt + 1) * 128,
                            ],
                            rhs=qtg[b][g][32 * j : 32 * j + HD, cs],
                            start=True,
                            stop=True,
                        )
                    e = ep.tile([128, NCHUNK * nh], bf16, name="e", tag="e")
                    nc.scalar.activation(e, sc, AF.Exp)
                    ev = e.rearrange("p (j n) -> p j n", j=nh)
                    m1 = mask_sb[b][
                        :, mt * N + c * NCHUNK : mt * N + (c + 1) * NCHUNK
                    ]
                    nc.vector.tensor_mul(
                        ev, ev, m1[:, None, :].to_broadcast([128, nh, NCHUNK])
                    )
                    if mt >= 1 and nh == 3:
                        # spread side work (projections for upcoming
                        # batches/groups) through nh=3 blocks only: the
                        # nh=2 blocks have the shortest exp period and no
                        # VectorE slack for extra casts. The masks stay
                        # first in the VectorE FIFO (they gate the lag-2
                        # A@V chain); piece casts ride behind them.
                        bi = (b * 2 + c) * 3 + g
                        npop = 2 if bi == 0 else (1 if bi == 1 else mt % 2)
                        for _ in range(npop):
                            if workq:
                                workq.pop(0)()
                        if mt == 6 and outq:
                            # out-projection of the previous chunk: by
                            # iteration 6 its normalization inputs are
                            # long done, so its matmuls never stall the
                            # PE queue.
                            outq.pop(0)()
                    e_live[mt] = e
                    if mt == MT - 1:
                        last = make_av(mt)
                        avq.append(lambda: (last(), emit_norm()))
                    else:
                        avq.append(make_av(mt))
                    pump_avq()

            def emit_outproj(b, c):
                def emit():
                    cs = slice(c * NCHUNK, (c + 1) * NCHUNK)
                    op = pscore.tile([128, NCHUNK], f32, name="op_ps", tag="sc")
                    for g in range(3):
                        k = 32 * len(GROUPS[g])
                        nc.tensor.matmul(
                            op,
                            lhsT=wo_sb[g][:k, :],
                            rhs=headsT[b][g][:k, cs],
                            start=(g == 0),
                            stop=(g == 2),
                        )
                    ost = sp.tile([128, NCHUNK], f32, name="ostage")
                    nc.vector.tensor_copy(ost, op)
                    nc.sync.dma_start(out_d[b, :, cs], ost)

                outq.append(emit)

            # ---------- emission schedule ----------
            # minimal critical path to the first exp: qt DMA -> g0
            # projections + the first half of V; everything else spreads
            # through the work queue.
            for p in projqk_pieces(0, 0, use_scalar=True):
                p()
            pv0 = projv_pieces(0)
            for p in pv0[:5]:  # alloc + m-tiles 0-3
                p()
            # group-1 projections also run in the prologue with their
            # casts on the still-idle ScalarE: frees batch-0 VectorE
            # (nearly saturated by masks + batch-1 prep) of ~3us.
            for p in projqk_pieces(0, 1, use_scalar=True):
                p()

            workq = []
            outq = []
            workq += pv0[5:]  # V m-tiles 4-7 (needed from iteration 5)
            workq.append(lambda: emit_dmas(1))
            workq += projqk_pieces(0, 2, False)
            workq += projqk_pieces(1, 0, False)
            workq += projv_pieces(1)
            workq += projqk_pieces(1, 1, False)
            workq += projqk_pieces(1, 2, False)

            for b in range(BPC):
                for c in range(N // NCHUNK):
                    for g in range(3):
                        emit_block(b, c, g)
                    emit_outproj(b, c)
            while avq:
                avq.pop(0)()
            while workq:
                workq.pop(0)()
            while outq:
                outq.pop(0)()

    nc.compile()
    return nc


def _prep_weights(W_query, W_key, W_val, W_out):
    wq = np.zeros((3, D, 128), np.float16)
    wk = np.zeros((3, D, 128), np.float16)
    wo = np.zeros((3, 128, D), np.float16)
    for g, heads in enumerate(GROUPS):
        for j, h in enumerate(heads):
            wq[g, :, 32 * j : 32 * j + HD] = W_query[h] * NORM
            wk[g, :, 32 * j : 32 * j + HD] = W_key[h]
            wo[g, 32 * j : 32 * j + HD, :] = W_out[h]
    wv = np.zeros((D, H * VSTRIDE), np.float16)
    for h in range(H):
        wv[:, h * VSTRIDE : h * VSTRIDE + HD] = W_val[h]
    return wq, wk, wv, wo


last_results = None


def kernel(**inputs):
    from concourse.bass_utils import run_bass_kernel_spmd

    q = np.ascontiguousarray(np.asarray(inputs["q"], dtype=np.float32))
    mask = np.asarray(inputs["mask"])
    W_query = np.asarray(inputs["W_query"], dtype=np.float32)
    W_key = np.asarray(inputs["W_key"], dtype=np.float32)
    W_val = np.asarray(inputs["W_val"], dtype=np.float32)
    W_out = np.asarray(inputs["W_out"], dtype=np.float32)

    qT = np.ascontiguousarray(q.transpose(0, 2, 1).astype(np.float16))  # [B, D, N]
    import ml_dtypes

    maskT = np.ascontiguousarray(
        mask.transpose(0, 2, 1).astype(ml_dtypes.bfloat16)
    )  # [B, N(m), N(n)]
    wq, wk, wv, wo = _prep_weights(W_query, W_key, W_val, W_out)

    if "nc" not in _built:
        _built["nc"] = _build_nc()
    nc = _built["nc"]

    in_maps = []
    for ci in range(NCORES):
        bs = slice(ci * BPC, (ci + 1) * BPC)
        in_maps.append(
            {
                "qt": np.ascontiguousarray(qT[bs]),
                "maskt": np.ascontiguousarray(maskT[bs]),
                "wq": wq,
                "wk": wk,
                "wv": wv,
                "wo": wo,
            }
        )

    global last_results
    res = run_bass_kernel_spmd(
        nc,
        in_maps,
        core_ids=list(range(NCORES)),
        trace=bool(int(os.environ.get("MHA_TRACE", "0"))),
    )
    last_results = res
    if res.exec_time_ns is not None:
        print(f"HW exec time: {res.exec_time_ns} ns")
    # outT is [BPC, D, N] per core; transpose back to [b, n, e]
    out = np.concatenate(
        [r["outT"].transpose(0, 2, 1) for r in res.results], axis=0
    )
    return np.ascontiguousarray(out.astype(np.float32))


# revision 36
# speedup vs baseline: 1.0102x; 1.0102x over previous
"""Multi-head self-attention Bass/Tile kernel for Trainium2 (8 NeuronCores).

Problem: B=16, N=1024, input/embed dim 128, 8 heads x head_dim 16.
  Q = q @ Wq_h; K = q @ Wk_h; V = q @ Wv_h   (per head)
  S = norm * Q K^T, masked softmax over keys, out = sum_h (A_h V_h) @ Wo_h

Sharding: data-parallel over batch, 2 batches per core, no collectives.

Per-core design (transposed "S^T" layout: softmaxed weights live as
[key m on partitions, query n on free] so A@V needs no transposes):

  * host sends qT [d, n] fp16 and maskT [m, n] bf16; per-head weights
    packed into 32-row-strip group layouts (3/3/2 heads per group).
  * projections: QT_g/KT_g [128, n] fp16 with head j of a group at
    partition strip 32j; V in natural [m, (h,v)] bf16 with an appended
    ones column per head (computes the softmax denominator during A@V).
  * steady loop per (chunk c, group g): for each key m-tile:
    scores via row-tiled fp16 matmuls (K=16, concurrent in 32-row PE
    strips) into one multi-bank PSUM tile [128, 512*nh]; one ScalarE
    exp (PSUM->SBUF bf16); one VectorE mask multiply; A@V via col-tiled
    fp16 matmuls (M=17) accumulating into a single PSUM bank using
    start=True has_written semantics (no per-block memset).
  * normalization: VectorE stream_shuffle broadcasts each strip's
    denominator row (local row 16) to all 32 rows of its quadrant in
    one op, reciprocal, then one multiply into headsT (fp16).
  * output projection transposed: outT[e, n] = sum_g Wo_g^T @ headsT_g
    (3 accumulating matmuls per 512-chunk); host transposes back.
  * software pipelining: the last A@V + normalization + out-projection
    of a block are emitted after the next block's first scores/exp so
    the ScalarE exp stream (the bottleneck) never stalls; batch 1's
    projections are interleaved into batch 0's blocks.
"""

import os
import sys

sys.path.insert(0, "/opt/trn_rl_repo")

import numpy as np

B, N, D, H, HD = 16, 1024, 128, 8, 16
NCORES = 8
BPC = B // NCORES  # batches per core
NORM = 1.0 / np.sqrt(HD)
GROUPS = [(0, 1, 2), (3, 4, 5), (6, 7)]
NCHUNK = 512  # query free-dim chunk (one PSUM bank)
MT = N // 128  # key m-tiles per batch
VSTRIDE = HD + 1  # V columns per head incl. ones column

SIM_MODE = False  # kept for harness compatibility; unused

_built = {}


def _build_nc():
    import concourse.mybir as mybir
    from concourse import bacc
    from concourse.tile import TileContext

    f32 = mybir.dt.float32
    f16 = mybir.dt.float16
    bf16 = mybir.dt.bfloat16
    AF = mybir.ActivationFunctionType

    nc = bacc.Bacc()

    qt_d = nc.dram_tensor("qt", [BPC, D, N], f16, kind="ExternalInput")
    mk_d = nc.dram_tensor("maskt", [BPC, N, N], bf16, kind="ExternalInput")
    wq_d = nc.dram_tensor("wq", [3, D, 128], f16, kind="ExternalInput")
    wk_d = nc.dram_tensor("wk", [3, D, 128], f16, kind="ExternalInput")
    wv_d = nc.dram_tensor("wv", [D, H * VSTRIDE], f16, kind="ExternalInput")
    wo_d = nc.dram_tensor("wo", [3, 128, D], f16, kind="ExternalInput")
    out_d = nc.dram_tensor("outT", [BPC, D, N], f32, kind="ExternalOutput")

    with TileContext(nc) as tc:
        with (
            tc.sbuf_pool(name="consts", bufs=1) as consts,
            tc.sbuf_pool(name="perbatch", bufs=2) as pb,
            tc.sbuf_pool(name="epool", bufs=6) as ep,
            tc.sbuf_pool(name="dpool", bufs=2) as dp,
            tc.sbuf_pool(name="spool", bufs=2) as sp,
            tc.psum_pool(name="pscore", bufs=2) as pscore,
            tc.psum_pool(name="pwork", bufs=2) as pwork,
        ):
            # exp table preload: make the first ACTIVATE (and its ~2.7us
            # table load) happen during the initial DMAs, off the
            # critical path.
            warm = consts.tile([1, 16], f32, name="warm")
            nc.vector.memset(warm, 0.0)
            nc.scalar.activation(warm, warm, AF.Exp)

            # zero row for the K=1 bank-zeroing matmuls
            zc = consts.tile([1, NCHUNK], bf16, name="zc")
            nc.vector.memset(zc, 0.0)

            # ---------- per-batch state ----------
            qt_sb = {}
            mask_sb = {}
            qtg = {}
            ktg = {}
            vaug = {}
            headsT = {}

            def emit_qt(b):
                qt_sb[b] = pb.tile([D, N], f16, name="qt_sb")
                for h in range(8):
                    hs = slice(h * (N // 8), (h + 1) * (N // 8))
                    nc.sync.dma_start(qt_sb[b][:, hs], qt_d[b][:, hs])

            def emit_masks(b):
                mask_sb[b] = pb.tile([128, MT * N], bf16, name="mask_sb")
                for mt in range(MT):
                    nc.sync.dma_start(
                        mask_sb[b][:, mt * N : (mt + 1) * N],
                        mk_d[b, mt * 128 : (mt + 1) * 128, :],
                    )

            def emit_dmas(b):
                emit_qt(b)
                emit_masks(b)

            # --- weights: everything on the critical path to the first
            # exp (g0 weights, qt, wv) clears the sync queue before the
            # bulky mask transfers; remaining weights follow.
            wq_sb, wk_sb, wo_sb = [None] * 3, [None] * 3, [None] * 3

            def load_w(lst, idx, shape, name, src):
                t = consts.tile(shape, f16, name=name)
                nc.sync.dma_start(t, src)
                lst[idx] = t

            load_w(wq_sb, 0, [D, 128], "wq_sb0", wq_d[0])
            load_w(wk_sb, 0, [D, 128], "wk_sb0", wk_d[0])
            emit_qt(0)
            wv_sb = consts.tile([D, H * VSTRIDE], f16, name="wv_sb")
            nc.sync.dma_start(wv_sb, wv_d[:, :])
            emit_masks(0)
            for g in (1, 2):
                load_w(wq_sb, g, [D, 128], f"wq_sb{g}", wq_d[g])
                load_w(wk_sb, g, [D, 128], f"wk_sb{g}", wk_d[g])
            for g in range(3):
                load_w(wo_sb, g, [128, D], f"wo_sb{g}", wo_d[g])

            # PE warm-up during the input DMA wait: a few long matmuls
            # flip the HAM clock gate to 8/8 so the projection matmuls
            # and first blocks run at 2.4 GHz.
            wup = pwork.tile([128, NCHUNK], f32, name="wup", tag="w")
            for _ in range(6):
                nc.tensor.matmul(
                    wup, lhsT=zc[:, :128], rhs=zc[:, :], start=True, stop=True
                )

            def projqk_pieces(b, g, use_scalar):
                # Q/K projections for group g as granular (matmul, cast)
                # pieces so the work queue can spread them one per
                # m-tile iteration. use_scalar routes the Q casts to
                # ScalarE when it would otherwise idle (batch 0 head).
                def alloc():
                    if g == 0:
                        qtg[b] = {}
                        ktg[b] = {}
                    qtg[b][g] = pb.tile([128, N], f16, name=f"qtg{g}")
                    ktg[b][g] = pb.tile([128, N], f16, name=f"ktg{g}")

                def one(c, which):
                    cs = slice(c * NCHUNK, (c + 1) * NCHUNK)
                    if which == "q":
                        ps = pscore.tile([128, NCHUNK], f32, name="ps_q", tag="sc")
                        nc.tensor.matmul(
                            ps, lhsT=wq_sb[g], rhs=qt_sb[b][:, cs],
                            start=True, stop=True,
                        )
                        if use_scalar:
                            nc.scalar.copy(qtg[b][g][:, cs], ps)
                        else:
                            nc.vector.tensor_copy(qtg[b][g][:, cs], ps)
                    else:
                        ps2 = pscore.tile([128, NCHUNK], f32, name="ps_k", tag="sc")
                        nc.tensor.matmul(
                            ps2, lhsT=wk_sb[g], rhs=qt_sb[b][:, cs],
                            start=True, stop=True,
                        )
                        nc.vector.tensor_copy(ktg[b][g][:, cs], ps2)

                pieces = [alloc]
                for c in range(N // NCHUNK):
                    pieces.append(lambda c=c: one(c, "q"))
                    pieces.append(lambda c=c: one(c, "k"))
                return pieces

            def projv_pieces(b):
                def alloc():
                    vaug[b] = pb.tile([128, MT * H * VSTRIDE], bf16, name="vaug")
                    vview = vaug[b].rearrange(
                        "p (mt h s) -> p mt h s", mt=MT, h=H, s=VSTRIDE
                    )
                    # ones columns first; the copies below skip them, so
                    # an A@V for m-tile mt depends only on copy(mt).
                    nc.vector.memset(vview[:, :, :, HD : HD + 1], 1.0)
                    headsT[b] = []
                    for g in range(3):
                        headsT[b].append(pb.tile([128, N], f16, name=f"headsT{g}"))

                def one(mt):
                    vview = vaug[b].rearrange(
                        "p (mt h s) -> p mt h s", mt=MT, h=H, s=VSTRIDE
                    )
                    ps = pscore.tile([128, NCHUNK], f32, name="ps_v", tag="sc")
                    nc.tensor.matmul(
                        ps[:, : H * VSTRIDE],
                        lhsT=qt_sb[b][:, mt * 128 : (mt + 1) * 128],
                        rhs=wv_sb,
                        start=True,
                        stop=True,
                    )
                    psv = ps[:, : H * VSTRIDE].rearrange(
                        "p (h s) -> p h s", h=H, s=VSTRIDE
                    )
                    nc.vector.tensor_copy(vview[:, mt, :, :HD], psv[:, :, :HD])

                return [alloc] + [lambda mt=mt: one(mt) for mt in range(MT)]

            # deferred work (last AV + normalization of the previous
            # block, out-projection of the previous chunk), emitted
            # Global lag-2 A@V pipeline: the A@V emitted in an iteration
            # is the one from TWO iterations back, so the mask multiply
            # it depends on is two exp-periods old and never stalls the
            # in-order PE queue (which would head-of-line-block the next
            # scores and starve the exp stream). avq carries the
            # not-yet-emitted A@V closures across block boundaries.
            avq = []

            def pump_avq():
                if len(avq) > 2:
                    avq.pop(0)()

            def emit_block(b, c, g):
                heads = GROUPS[g]
                nh = len(heads)
                cs = slice(c * NCHUNK, (c + 1) * NCHUNK)
                av = pwork.tile([128, NCHUNK], f32, name="av_ps", tag="w")
                # K=1 zeroing matmul: opens the bank's accumulation group
                # and initializes every element, so the col-tiled AV
                # matmuls below can accumulate (identical semantics on HW
                # and in CoreSim's pending-zero model).
                nc.tensor.matmul(
                    av, lhsT=zc[:, :128], rhs=zc[:, :], start=True, stop=False,
                    skip_group_check=True,
                )
                e_live = {}

                def make_av(mt):
                    def emit_av():
                        ep_ = e_live.pop(mt)
                        for j in range(nh):
                            h = heads[j]
                            nc.tensor.matmul(
                                av[32 * j : 32 * j + VSTRIDE, :],
                                lhsT=vaug[b][
                                    :,
                                    mt * H * VSTRIDE
                                    + h * VSTRIDE : mt * H * VSTRIDE
                                    + (h + 1) * VSTRIDE,
                                ],
                                rhs=ep_[:, j * NCHUNK : (j + 1) * NCHUNK],
                                start=False,
                                stop=(mt == MT - 1 and j == nh - 1),
                                skip_group_check=True,
                            )

                    return emit_av

                def emit_norm():
                    # evacuate av to SBUF (releases the PSUM slot), then
                    # broadcast each strip's denominator row (local row
                    # 16) across its 32-row quadrant via stream_shuffle,
                    # reciprocal, and scale into headsT.
                    ds = dp.tile([96, NCHUNK], f32, name="dsrc")
                    nc.vector.tensor_copy(ds, av[0:96, :])
                    dn = dp.tile([96, NCHUNK], f32, name="dnorm")
                    nc.vector.stream_shuffle(dn, ds, mask=[16] * 32)
                    nc.vector.reciprocal_approx_fast(dn, dn)
                    # all-SBUF multiply: run it on the otherwise-idle
                    # GpSimd to keep VectorE under the exp-period budget
                    nc.gpsimd.tensor_mul(headsT[b][g][0:96, cs], ds, dn)

                for mt in range(MT):
                    sc = pscore.tile([128, NCHUNK * nh], f32, name="sc", tag="sc")
                    for j in range(nh):
                        nc.tensor.matmul(
                            sc[:, j * NCHUNK : (j + 1) * NCHUNK],
                            lhsT=ktg[b][g][
                                32 * j : 32 * j + HD,
                                mt * 128 : (mt + 1) * 128,
                            ],
                            rhs=qtg[b][g][32 * j : 32 * j + HD, cs],
                            start=True,
                            stop=True,
                        )
                    e = ep.tile([128, NCHUNK * nh], bf16, name="e", tag="e")
                    nc.scalar.activation(e, sc, AF.Exp)
                    if mt >= 1:
                        # spread side work (projections for upcoming
                        # batches/groups) a piece at a time; early blocks
                        # drain faster (their pieces are needed sooner),
                        # later blocks take one every other iteration,
                        # pushing batch-1's group projections into
                        # batch-1's own slack.
                        bi = (b * 2 + c) * 3 + g
                        npop = 2 if bi == 0 else (1 if bi == 1 else mt % 2)
                        for _ in range(npop):
                            if workq:
                                workq.pop(0)()
                        if mt == 6 and outq:
                            # out-projection of the previous chunk: by
                            # iteration 6 its normalization inputs are
                            # long done, so its matmuls never stall the
                            # PE queue.
                            outq.pop(0)()
                    ev = e.rearrange("p (j n) -> p j n", j=nh)
                    m1 = mask_sb[b][
                        :, mt * N + c * NCHUNK : mt * N + (c + 1) * NCHUNK
                    ]
                    nc.vector.tensor_mul(
                        ev, ev, m1[:, None, :].to_broadcast([128, nh, NCHUNK])
                    )
                    e_live[mt] = e
                    if mt == MT - 1:
                        last = make_av(mt)
                        avq.append(lambda: (last(), emit_norm()))
                    else:
                        avq.append(make_av(mt))
                    pump_avq()

            def emit_outproj(b, c):
                def emit():
                    cs = slice(c * NCHUNK, (c + 1) * NCHUNK)
                    op = pscore.tile([128, NCHUNK], f32, name="op_ps", tag="sc")
                    for g in range(3):
                        k = 32 * len(GROUPS[g])
                        nc.tensor.matmul(
                            op,
                            lhsT=wo_sb[g][:k, :],
                            rhs=headsT[b][g][:k, cs],
                            start=(g == 0),
                            stop=(g == 2),
                        )
                    ost = sp.tile([128, NCHUNK], f32, name="ostage")
                    nc.vector.tensor_copy(ost, op)
                    nc.sync.dma_start(out_d[b, :, cs], ost)

                outq.append(emit)

            # ---------- emission schedule ----------
            # minimal critical path to the first exp: qt DMA -> g0
            # projections + the first half of V; everything else spreads
            # through the work queue.
            for p in projqk_pieces(0, 0, use_scalar=True):
                p()
            pv0 = projv_pieces(0)
            for p in pv0[:5]:  # alloc + m-tiles 0-3
                p()
            workq = []
            outq = []
            workq += pv0[5:]  # V m-tiles 4-7 (needed from iteration 5)
            workq.append(lambda: emit_dmas(1))
            workq += projqk_pieces(0, 1, False)
            workq += projqk_pieces(0, 2, False)
            workq += projqk_pieces(1, 0, False)
            workq += projv_pieces(1)
            workq += projqk_pieces(1, 1, False)
            workq += projqk_pieces(1, 2, False)

            for b in range(BPC):
                for c in range(N // NCHUNK):
                    for g in range(3):
                        emit_block(b, c, g)
                    emit_outproj(b, c)
            while avq:
                avq.pop(0)()
            while workq:
                workq.pop(0)()
            while outq:
                outq.pop(0)()

    nc.compile()
    return nc


def _prep_weights(W_query, W_key, W_val, W_out):
    wq = np.zeros((3, D, 128), np.float16)
    wk = np.zeros((3, D, 128), np.float16)
    wo = np.zeros((3, 128, D), np.float16)
    for g, heads in enumerate(GROUPS):
        for j, h in enumerate(heads):
            wq[g, :, 32 * j : 32 * j + HD] = W_query[h] * NORM
            wk[g, :, 32 * j : 32 * j + HD] = W_key[h]
            wo[g, 32 * j : 32 * j + HD, :] = W_out[h]
    wv = np.zeros((D, H * VSTRIDE), np.float16)
    for h in range(H):
        wv[:, h * VSTRIDE : h * VSTRIDE + HD] = W_val[h]
    return wq, wk, wv, wo


last_results = None


def kernel(**inputs):
    from concourse.bass_utils import run_bass_kernel_spmd

    q = np.ascontiguousarray(np.asarray(inputs["q"], dtype=np.float32))
    mask = np.asarray(inputs["mask"])
    W_query = np.asarray(inputs["W_query"], dtype=np.float32)
    W_key = np.asarray(inputs["W_key"], dtype=np.float32)
    W_val = np.asarray(inputs["W_val"], dtype=np.float32)
    W_out = np.asarray(inputs["W_out"], dtype=np.float32)

    qT = np.ascontiguousarray(q.transpose(0, 2, 1).astype(np.float16))  # [B, D, N]
    import ml_dtypes

    maskT = np.ascontiguousarray(
        mask.transpose(0, 2, 1).astype(ml_dtypes.bfloat16)
    )  # [B, N(m), N(n)]
    wq, wk, wv, wo = _prep_weights(W_query, W_key, W_val, W_out)

    if "nc" not in _built:
        _built["nc"] = _build_nc()
    nc = _built["nc"]

    in_maps = []
    for ci in range(NCORES):
        bs = slice(ci * BPC, (ci + 1) * BPC)
        in_maps.append(
            {
                "qt": np.ascontiguousarray(qT[bs]),
                "maskt": np.ascontiguousarray(maskT[bs]),
                "wq": wq,
                "wk": wk,
                "wv": wv,
                "wo": wo,
            }
        )

    global last_results
    res = run_bass_kernel_spmd(
        nc,
        in_maps,
        core_ids=list(range(NCORES)),
        trace=bool(int(os.environ.get("MHA_TRACE", "0"))),
    )
    last_results = res
    if res.exec_time_ns is not None:
        print(f"HW exec time: {res.exec_time_ns} ns")
    # outT is [BPC, D, N] per core; transpose back to [b, n, e]
    out = np.concatenate(
        [r["outT"].transpose(0, 2, 1) for r in res.results], axis=0
    )
    return np.ascontiguousarray(out.astype(np.float32))


# revision 37
# speedup vs baseline: 1.0269x; 1.0165x over previous
"""Multi-head self-attention Bass/Tile kernel for Trainium2 (8 NeuronCores).

Problem: B=16, N=1024, input/embed dim 128, 8 heads x head_dim 16.
  Q = q @ Wq_h; K = q @ Wk_h; V = q @ Wv_h   (per head)
  S = norm * Q K^T, masked softmax over keys, out = sum_h (A_h V_h) @ Wo_h

Sharding: data-parallel over batch, 2 batches per core, no collectives.

Per-core design (transposed "S^T" layout: softmaxed weights live as
[key m on partitions, query n on free] so A@V needs no transposes):

  * host sends qT [d, n] fp16 and maskT [m, n] bf16; per-head weights
    packed into 32-row-strip group layouts (3/3/2 heads per group).
  * projections: QT_g/KT_g [128, n] fp16 with head j of a group at
    partition strip 32j; V in natural [m, (h,v)] bf16 with an appended
    ones column per head (computes the softmax denominator during A@V).
  * steady loop per (chunk c, group g): for each key m-tile:
    scores via row-tiled fp16 matmuls (K=16, concurrent in 32-row PE
    strips) into one multi-bank PSUM tile [128, 512*nh]; one ScalarE
    exp (PSUM->SBUF bf16); one VectorE mask multiply; A@V via col-tiled
    fp16 matmuls (M=17) accumulating into a single PSUM bank using
    start=True has_written semantics (no per-block memset).
  * normalization: VectorE stream_shuffle broadcasts each strip's
    denominator row (local row 16) to all 32 rows of its quadrant in
    one op, reciprocal, then one multiply into headsT (fp16).
  * output projection transposed: outT[e, n] = sum_g Wo_g^T @ headsT_g
    (3 accumulating matmuls per 512-chunk); host transposes back.
  * software pipelining: the last A@V + normalization + out-projection
    of a block are emitted after the next block's first scores/exp so
    the ScalarE exp stream (the bottleneck) never stalls; batch 1's
    projections are interleaved into batch 0's blocks.
"""

import os
import sys

sys.path.insert(0, "/opt/trn_rl_repo")

import numpy as np

B, N, D, H, HD = 16, 1024, 128, 8, 16
NCORES = 8
BPC = B // NCORES  # batches per core
NORM = 1.0 / np.sqrt(HD)
GROUPS = [(0, 1, 2), (3, 4, 5), (6, 7)]
NCHUNK = 512  # query free-dim chunk (one PSUM bank)
MT = N // 128  # key m-tiles per batch
VSTRIDE = HD + 1  # V columns per head incl. ones column

SIM_MODE = False  # kept for harness compatibility; unused

_built = {}


def _build_nc():
    import concourse.mybir as mybir
    from concourse import bacc
    from concourse.tile import TileContext

    f32 = mybir.dt.float32
    f16 = mybir.dt.float16
    bf16 = mybir.dt.bfloat16
    AF = mybir.ActivationFunctionType

    nc = bacc.Bacc()

    qt_d = nc.dram_tensor("qt", [BPC, D, N], f16, kind="ExternalInput")
    mk_d = nc.dram_tensor("maskt", [BPC, N, N], bf16, kind="ExternalInput")
    wq_d = nc.dram_tensor("wq", [3, D, 128], f16, kind="ExternalInput")
    wk_d = nc.dram_tensor("wk", [3, D, 128], f16, kind="ExternalInput")
    wv_d = nc.dram_tensor("wv", [D, H * VSTRIDE], f16, kind="ExternalInput")
    wo_d = nc.dram_tensor("wo", [3, 128, D], f16, kind="ExternalInput")
    out_d = nc.dram_tensor("outT", [BPC, D, N], bf16, kind="ExternalOutput")

    with TileContext(nc) as tc:
        with (
            tc.sbuf_pool(name="consts", bufs=1) as consts,
            tc.sbuf_pool(name="perbatch", bufs=2) as pb,
            tc.sbuf_pool(name="epool", bufs=6) as ep,
            tc.sbuf_pool(name="dpool", bufs=2) as dp,
            tc.sbuf_pool(name="spool", bufs=2) as sp,
            tc.psum_pool(name="pscore", bufs=2) as pscore,
            tc.psum_pool(name="pwork", bufs=2) as pwork,
        ):
            # exp table preload: make the first ACTIVATE (and its ~2.7us
            # table load) happen during the initial DMAs, off the
            # critical path.
            warm = consts.tile([1, 16], f32, name="warm")
            nc.vector.memset(warm, 0.0)
            nc.scalar.activation(warm, warm, AF.Exp)

            # zero row for the K=1 bank-zeroing matmuls
            zc = consts.tile([1, NCHUNK], bf16, name="zc")
            nc.vector.memset(zc, 0.0)

            # ---------- per-batch state ----------
            qt_sb = {}
            mask_sb = {}
            qtg = {}
            ktg = {}
            vaug = {}
            headsT = {}

            def emit_qt(b):
                qt_sb[b] = pb.tile([D, N], f16, name="qt_sb")
                for h in range(4):
                    hs = slice(h * (N // 4), (h + 1) * (N // 4))
                    nc.sync.dma_start(qt_sb[b][:, hs], qt_d[b][:, hs])

            def emit_masks(b):
                mask_sb[b] = pb.tile([128, MT * N], bf16, name="mask_sb")
                for mt in range(MT):
                    nc.sync.dma_start(
                        mask_sb[b][:, mt * N : (mt + 1) * N],
                        mk_d[b, mt * 128 : (mt + 1) * 128, :],
                    )

            def emit_dmas(b):
                emit_qt(b)
                emit_masks(b)

            # --- weights: everything on the critical path to the first
            # exp (g0 weights, qt, wv) clears the sync queue before the
            # bulky mask transfers; remaining weights follow.
            wq_sb, wk_sb, wo_sb = [None] * 3, [None] * 3, [None] * 3

            def load_w(lst, idx, shape, name, src):
                t = consts.tile(shape, f16, name=name)
                nc.sync.dma_start(t, src)
                lst[idx] = t

            load_w(wq_sb, 0, [D, 128], "wq_sb0", wq_d[0])
            load_w(wk_sb, 0, [D, 128], "wk_sb0", wk_d[0])
            emit_qt(0)
            wv_sb = consts.tile([D, H * VSTRIDE], f16, name="wv_sb")
            nc.sync.dma_start(wv_sb, wv_d[:, :])
            emit_masks(0)
            for g in (1, 2):
                load_w(wq_sb, g, [D, 128], f"wq_sb{g}", wq_d[g])
                load_w(wk_sb, g, [D, 128], f"wk_sb{g}", wk_d[g])
            for g in range(3):
                load_w(wo_sb, g, [128, D], f"wo_sb{g}", wo_d[g])

            # PE warm-up during the input DMA wait: a few long matmuls
            # flip the HAM clock gate to 8/8 so the projection matmuls
            # and first blocks run at 2.4 GHz.
            wup = pwork.tile([128, NCHUNK], f32, name="wup", tag="w")
            for _ in range(6):
                nc.tensor.matmul(
                    wup, lhsT=zc[:, :128], rhs=zc[:, :], start=True, stop=True
                )

            def projqk_pieces(b, g, use_scalar):
                # Q/K projections for group g as granular (matmul, cast)
                # pieces so the work queue can spread them one per
                # m-tile iteration. use_scalar routes the Q casts to
                # ScalarE when it would otherwise idle (batch 0 head).
                def alloc():
                    if g == 0:
                        qtg[b] = {}
                        ktg[b] = {}
                    qtg[b][g] = pb.tile([128, N], f16, name=f"qtg{g}")
                    ktg[b][g] = pb.tile([128, N], f16, name=f"ktg{g}")

                def one(c, which):
                    cs = slice(c * NCHUNK, (c + 1) * NCHUNK)
                    if which == "q":
                        ps = pscore.tile([128, NCHUNK], f32, name="ps_q", tag="sc")
                        nc.tensor.matmul(
                            ps, lhsT=wq_sb[g], rhs=qt_sb[b][:, cs],
                            start=True, stop=True,
                        )
                        if use_scalar:
                            nc.scalar.copy(qtg[b][g][:, cs], ps)
                        else:
                            nc.vector.tensor_copy(qtg[b][g][:, cs], ps)
                    else:
                        ps2 = pscore.tile([128, NCHUNK], f32, name="ps_k", tag="sc")
                        nc.tensor.matmul(
                            ps2, lhsT=wk_sb[g], rhs=qt_sb[b][:, cs],
                            start=True, stop=True,
                        )
                        nc.vector.tensor_copy(ktg[b][g][:, cs], ps2)

                pieces = [alloc]
                for c in range(N // NCHUNK):
                    pieces.append(lambda c=c: one(c, "q"))
                    pieces.append(lambda c=c: one(c, "k"))
                return pieces

            def projv_pieces(b):
                def alloc():
                    vaug[b] = pb.tile([128, MT * H * VSTRIDE], bf16, name="vaug")
                    vview = vaug[b].rearrange(
                        "p (mt h s) -> p mt h s", mt=MT, h=H, s=VSTRIDE
                    )
                    # ones columns first; the copies below skip them, so
                    # an A@V for m-tile mt depends only on copy(mt).
                    nc.vector.memset(vview[:, :, :, HD : HD + 1], 1.0)
                    headsT[b] = []
                    for g in range(3):
                        headsT[b].append(pb.tile([128, N], f16, name=f"headsT{g}"))

                def one(mt):
                    vview = vaug[b].rearrange(
                        "p (mt h s) -> p mt h s", mt=MT, h=H, s=VSTRIDE
                    )
                    ps = pscore.tile([128, NCHUNK], f32, name="ps_v", tag="sc")
                    nc.tensor.matmul(
                        ps[:, : H * VSTRIDE],
                        lhsT=qt_sb[b][:, mt * 128 : (mt + 1) * 128],
                        rhs=wv_sb,
                        start=True,
                        stop=True,
                    )
                    psv = ps[:, : H * VSTRIDE].rearrange(
                        "p (h s) -> p h s", h=H, s=VSTRIDE
                    )
                    nc.vector.tensor_copy(vview[:, mt, :, :HD], psv[:, :, :HD])

                return [alloc] + [lambda mt=mt: one(mt) for mt in range(MT)]

            # deferred work (last AV + normalization of the previous
            # block, out-projection of the previous chunk), emitted
            # Global lag-2 A@V pipeline: the A@V emitted in an iteration
            # is the one from TWO iterations back, so the mask multiply
            # it depends on is two exp-periods old and never stalls the
            # in-order PE queue (which would head-of-line-block the next
            # scores and starve the exp stream). avq carries the
            # not-yet-emitted A@V closures across block boundaries.
            avq = []

            def pump_avq():
                if len(avq) > 2:
                    avq.pop(0)()

            def emit_block(b, c, g):
                heads = GROUPS[g]
                nh = len(heads)
                cs = slice(c * NCHUNK, (c + 1) * NCHUNK)
                av = pwork.tile([128, NCHUNK], f32, name="av_ps", tag="w")
                # K=1 zeroing matmul: opens the bank's accumulation group
                # and initializes every element, so the col-tiled AV
                # matmuls below can accumulate (identical semantics on HW
                # and in CoreSim's pending-zero model).
                nc.tensor.matmul(
                    av, lhsT=zc[:, :128], rhs=zc[:, :], start=True, stop=False,
                    skip_group_check=True,
                )
                e_live = {}

                def make_av(mt):
                    def emit_av():
                        ep_ = e_live.pop(mt)
                        for j in range(nh):
                            h = heads[j]
                            nc.tensor.matmul(
                                av[32 * j : 32 * j + VSTRIDE, :],
                                lhsT=vaug[b][
                                    :,
                                    mt * H * VSTRIDE
                                    + h * VSTRIDE : mt * H * VSTRIDE
                                    + (h + 1) * VSTRIDE,
                                ],
                                rhs=ep_[:, j * NCHUNK : (j + 1) * NCHUNK],
                                start=False,
                                stop=(mt == MT - 1 and j == nh - 1),
                                skip_group_check=True,
                            )

                    return emit_av

                def emit_norm():
                    # evacuate av to SBUF (releases the PSUM slot), then
                    # broadcast each strip's denominator row (local row
                    # 16) across its 32-row quadrant via stream_shuffle,
                    # reciprocal, and scale into headsT.
                    ds = dp.tile([96, NCHUNK], f32, name="dsrc")
                    nc.vector.tensor_copy(ds, av[0:96, :])
                    dn = dp.tile([96, NCHUNK], f32, name="dnorm")
                    nc.vector.stream_shuffle(dn, ds, mask=[16] * 32)
                    nc.vector.reciprocal_approx_fast(dn, dn)
                    # all-SBUF multiply: run it on the otherwise-idle
                    # GpSimd to keep VectorE under the exp-period budget
                    nc.gpsimd.tensor_mul(headsT[b][g][0:96, cs], ds, dn)

                for mt in range(MT):
                    sc = pscore.tile([128, NCHUNK * nh], f32, name="sc", tag="sc")
                    for j in range(nh):
                        nc.tensor.matmul(
                            sc[:, j * NCHUNK : (j + 1) * NCHUNK],
                            lhsT=ktg[b][g][
                                32 * j : 32 * j + HD,
                                mt * 128 : (mt + 1) * 128,
                            ],
                            rhs=qtg[b][g][32 * j : 32 * j + HD, cs],
                            start=True,
                            stop=True,
                        )
                    e = ep.tile([128, NCHUNK * nh], bf16, name="e", tag="e")
                    nc.scalar.activation(e, sc, AF.Exp)
                    if mt >= 1:
                        # spread side work (projections for upcoming
                        # batches/groups) a piece at a time; early blocks
                        # drain faster (their pieces are needed sooner),
                        # later blocks take one every other iteration,
                        # pushing batch-1's group projections into
                        # batch-1's own slack.
                        bi = (b * 2 + c) * 3 + g
                        npop = 2 if bi == 0 else (1 if bi == 1 else mt % 2)
                        for _ in range(npop):
                            if workq:
                                workq.pop(0)()
                        if mt == 6 and outq:
                            # out-projection of the previous chunk: by
                            # iteration 6 its normalization inputs are
                            # long done, so its matmuls never stall the
                            # PE queue.
                            outq.pop(0)()
                    ev = e.rearrange("p (j n) -> p j n", j=nh)
                    m1 = mask_sb[b][
                        :, mt * N + c * NCHUNK : mt * N + (c + 1) * NCHUNK
                    ]
                    nc.vector.tensor_mul(
                        ev, ev, m1[:, None, :].to_broadcast([128, nh, NCHUNK])
                    )
                    e_live[mt] = e
                    if mt == MT - 1:
                        last = make_av(mt)
                        avq.append(lambda: (last(), emit_norm()))
                    else:
                        avq.append(make_av(mt))
                    pump_avq()

            def emit_outproj(b, c):
                def emit():
                    cs = slice(c * NCHUNK, (c + 1) * NCHUNK)
                    op = pscore.tile([128, NCHUNK], f32, name="op_ps", tag="sc")
                    for g in range(3):
                        k = 32 * len(GROUPS[g])
                        nc.tensor.matmul(
                            op,
                            lhsT=wo_sb[g][:k, :],
                            rhs=headsT[b][g][:k, cs],
                            start=(g == 0),
                            stop=(g == 2),
                        )
                    ost = sp.tile([128, NCHUNK], bf16, name="ostage")
                    nc.vector.tensor_copy(ost, op)
                    for h in range(2):
                        hs = slice(h * (NCHUNK // 2), (h + 1) * (NCHUNK // 2))
                        nc.sync.dma_start(
                            out_d[b, :, c * NCHUNK + h * (NCHUNK // 2) :
                                  c * NCHUNK + (h + 1) * (NCHUNK // 2)],
                            ost[:, hs],
                        )

                outq.append(emit)

            # ---------- emission schedule ----------
            # minimal critical path to the first exp: qt DMA -> g0
            # projections + the first half of V; everything else spreads
            # through the work queue.
            for p in projqk_pieces(0, 0, use_scalar=True):
                p()
            pv0 = projv_pieces(0)
            for p in pv0[:5]:  # alloc + m-tiles 0-3
                p()
            workq = []
            outq = []
            workq += pv0[5:]  # V m-tiles 4-7 (needed from iteration 5)
            workq.append(lambda: emit_dmas(1))
            workq += projqk_pieces(0, 1, False)
            workq += projqk_pieces(0, 2, False)
            workq += projqk_pieces(1, 0, False)
            workq += projv_pieces(1)
            workq += projqk_pieces(1, 1, False)
            workq += projqk_pieces(1, 2, False)

            for b in range(BPC):
                for c in range(N // NCHUNK):
                    for g in range(3):
                        emit_block(b, c, g)
                    emit_outproj(b, c)
            while avq:
                avq.pop(0)()
            while workq:
                workq.pop(0)()
            while outq:
                outq.pop(0)()

    nc.compile()
    return nc


def _prep_weights(W_query, W_key, W_val, W_out):
    wq = np.zeros((3, D, 128), np.float16)
    wk = np.zeros((3, D, 128), np.float16)
    wo = np.zeros((3, 128, D), np.float16)
    for g, heads in enumerate(GROUPS):
        for j, h in enumerate(heads):
            wq[g, :, 32 * j : 32 * j + HD] = W_query[h] * NORM
            wk[g, :, 32 * j : 32 * j + HD] = W_key[h]
            wo[g, 32 * j : 32 * j + HD, :] = W_out[h]
    wv = np.zeros((D, H * VSTRIDE), np.float16)
    for h in range(H):
        wv[:, h * VSTRIDE : h * VSTRIDE + HD] = W_val[h]
    return wq, wk, wv, wo


last_results = None


def kernel(**inputs):
    from concourse.bass_utils import run_bass_kernel_spmd

    q = np.ascontiguousarray(np.asarray(inputs["q"], dtype=np.float32))
    mask = np.asarray(inputs["mask"])
    W_query = np.asarray(inputs["W_query"], dtype=np.float32)
    W_key = np.asarray(inputs["W_key"], dtype=np.float32)
    W_val = np.asarray(inputs["W_val"], dtype=np.float32)
    W_out = np.asarray(inputs["W_out"], dtype=np.float32)

    qT = np.ascontiguousarray(q.transpose(0, 2, 1).astype(np.float16))  # [B, D, N]
    import ml_dtypes

    maskT = np.ascontiguousarray(
        mask.transpose(0, 2, 1).astype(ml_dtypes.bfloat16)
    )  # [B, N(m), N(n)]
    wq, wk, wv, wo = _prep_weights(W_query, W_key, W_val, W_out)

    if "nc" not in _built:
        _built["nc"] = _build_nc()
    nc = _built["nc"]

    in_maps = []
    for ci in range(NCORES):
        bs = slice(ci * BPC, (ci + 1) * BPC)
        in_maps.append(
            {
                "qt": np.ascontiguousarray(qT[bs]),
                "maskt": np.ascontiguousarray(maskT[bs]),
                "wq": wq,
                "wk": wk,
                "wv": wv,
                "wo": wo,
            }
        )

    global last_results
    res = run_bass_kernel_spmd(
        nc,
        in_maps,
        core_ids=list(range(NCORES)),
        trace=bool(int(os.environ.get("MHA_TRACE", "0"))),
    )
    last_results = res
    if res.exec_time_ns is not None:
        print(f"HW exec time: {res.exec_time_ns} ns")
    # outT is [BPC, D, N] per core; transpose back to [b, n, e]
    out = np.concatenate(
        [r["outT"].transpose(0, 2, 1) for r in res.results], axis=0
    )
    return np.ascontiguousarray(out.astype(np.float32))
